# revision 14
# baseline (speedup 1.0000x reference)
"""BitNet decoder MLP on 8 Trainium2 NeuronCores (Bass/Tile).

Strategy: data-parallel over batch (512 rows/core).

Weights are ternary-quantized on device. Layer 0 is quantized locally on every
core from a full ob-major fp32 copy (its abs-mean is still computed
cooperatively + AllReduce), so L0 matmuls start as soon as the scale lands —
no AllGather on the critical path. Layers 1-3 are quantized cooperatively
(1/8 per core) into an fp8e4 image and AllGathered; collectives have a large
fixed cost and share one serial queue, so there is exactly one gather per
layer, ordered AR0 -> AR1 -> AG_L1 -> AR23 -> AG_L2 -> AG_L3.

The quantized image is fp8e4 (E4M3): ternary {-1,0,1} is exact in fp8, and
the tensor engine accepts mixed-dtype matmuls (bf16 stationary activations x
fp8 moving weights) at full rate, so the image is half the bytes of bf16 with
identical matmul time. All matmul arithmetic is exact: activations are
int8-valued bf16, weights {-1,0,1} fp8, accumulation fp32 in PSUM.

Engine budget: PE does matmuls (the 605us floor); ACT does abs passes, PSUM
eviction (dequant scale folded in) and the fused LN+SiLU (hw Silu table);
DVE does L0 quant, stats, act-quant; GpSimd does L1-3 quant + collectives;
transposes ride the DMA crossbar. Rounding via the fp32 magic-number trick
(round-half-even, matches jnp.round).
"""

import numpy as np

import concourse.bass as bass
import concourse.bass_isa as bass_isa
import concourse.mybir as mybir
import concourse.tile as tile
from concourse import bacc
from concourse.bass_utils import run_bass_kernel_spmd

F32 = mybir.dt.float32
BF16 = mybir.dt.bfloat16
FP8 = mybir.dt.float8e4
AF = mybir.ActivationFunctionType
OP = mybir.AluOpType

N_CORES = 8
P = 128
OBW = 512            # output block width (one PSUM bank of fp32)
CH_ELS = P * OBW     # elements per weight chunk
MAGIC = 12582912.0   # 1.5 * 2**23: fp32 round-to-nearest-even trick
EPS = 1e-5

FULL_CFG = dict(B=4096, D0=1024, H=4096, OBINS=1000)


def _plan(cfg):
    """Static per-layer plan."""
    B, D0, H, OBINS = cfg["B"], cfg["D0"], cfg["H"], cfg["OBINS"]
    o3_real = 2 * OBINS
    o3_pad = ((o3_real + OBW - 1) // OBW) * OBW
    dims = [
        dict(din=D0, dout=H, dreal=H),
        dict(din=H, dout=H, dreal=H),
        dict(din=H, dout=H, dreal=H),
        dict(din=H, dout=o3_pad, dreal=o3_real),
    ]
    numels = [H * D0, H * H, H * H, o3_real * H]  # real numels for mean|W|
    layers = []
    ch_base = 0
    for li, d in enumerate(dims):
        n_ic = d["din"] // P
        n_ob = d["dout"] // OBW
        n_ch = n_ob * n_ic
        assert n_ch % N_CORES == 0, (li, n_ch)
        panel_ic = min(8, n_ic, max(1, n_ch // N_CORES))
        assert n_ic % panel_ic == 0 and (n_ch // N_CORES) % panel_ic == 0
        n_panels = n_ic // panel_ic
        layers.append(dict(
            li=li, din=d["din"], dout=d["dout"], dreal=d["dreal"],
            n_ic=n_ic, n_ob=n_ob, n_ch=n_ch, per_rank=n_ch // N_CORES,
            panel_ic=panel_ic, n_panels=n_panels,
            numel=numels[li], ch_base=ch_base,
            ob_w=[min(OBW, d["dreal"] - ob * OBW) for ob in range(n_ob)],
        ))
        ch_base += n_ch
    total_ch = ch_base
    per_rank = total_ch // N_CORES
    b_core = B // N_CORES
    assert b_core % P == 0
    return layers, total_ch, per_rank, b_core // P


def _rsqrt_newton(nc, pool, v, n_iter=3):
    """istd = 1/sqrt(v) for v [128,1] fp32 (v > 0), pure-DVE Newton iteration.

    seed_bits = 0x5f370000 - bits(v)/2 computed in fp32 on aligned int32 views;
    the fp32 mantissa noise on the >2^24 intermediate is irrelevant for a seed.
    """
    seed = pool.tile([P, 1], F32, tag="rs_seed", name="rs_seed")
    seed_i32 = seed[:].bitcast(mybir.dt.int32)
    v_i32 = v.bitcast(mybir.dt.int32)
    nc.vector.tensor_scalar(seed_i32[:], v_i32[:], -0.5,
                            float(0x5F370000), OP.mult, OP.add)
    y = seed
    t1 = pool.tile([P, 1], F32, tag="rs_t1", name="rs_t1")
    t2 = pool.tile([P, 1], F32, tag="rs_t2", name="rs_t2")
    for _ in range(n_iter):
        nc.vector.tensor_tensor(t1[:], y[:], y[:], OP.mult)
        nc.vector.tensor_tensor(t2[:], t1[:], v, OP.mult)
        nc.vector.tensor_scalar(t1[:], t2[:], -0.5, 1.5, OP.mult, OP.add)
        nc.vector.tensor_tensor(y[:], y[:], t1[:], OP.mult)
    return y


def build(cfg):
    layers, total_ch, per_rank, T = _plan(cfg)
    nc = bacc.Bacc("TRN2", target_bir_lowering=False, debug=False,
                   num_devices=N_CORES)

    D0, OBINS = cfg["D0"], cfg["OBINS"]
    b_core = T * P
    L0 = layers[0]
    n_ch0 = L0["n_ch"]

    xs = nc.dram_tensor("xs", [b_core, D0], F32, kind="ExternalInput")
    # rank's weight chunks (all layers), unit-major flat fp32
    wsh = nc.dram_tensor("wsh", [per_rank * CH_ELS], F32, kind="ExternalInput")
    # full W0, ob-major unit layout (same copy on every core)
    w0f = nc.dram_tensor("w0f", [n_ch0 * CH_ELS], F32, kind="ExternalInput")
    mz_out = nc.dram_tensor("mz", [b_core, OBINS], F32, kind="ExternalOutput")
    ii_out = nc.dram_tensor("ii", [b_core, OBINS], F32, kind="ExternalOutput")

    with tile.TileContext(nc) as tc:
        with (
            tc.tile_pool(name="ybig", bufs=4) as ypool,        # 16KB/partition f32
            tc.tile_pool(name="wr", bufs=3) as wrpool,         # prep fp32 runs 8KB
            tc.tile_pool(name="w0", bufs=2) as w0pool,         # L0 quant src 8KB
            tc.tile_pool(name="wabs", bufs=2) as wabspool,     # L2/L3 abs runs 8KB
            tc.tile_pool(name="xqT", bufs=4) as xqTpool,       # [128,32,128] bf16
            tc.tile_pool(name="xqT0", bufs=4) as xqT0pool,     # [128,n_ic0,128] bf16
            tc.tile_pool(name="wp", bufs=2) as wpool,          # [128,16,512] fp8
            tc.tile_pool(name="wp0", bufs=2) as wp0pool,       # [128,n_ic0,512] fp8
            tc.tile_pool(name="xqn", bufs=2) as xqnpool,       # 8KB/partition bf16
            tc.tile_pool(name="u", bufs=2) as upool,           # [128,2048] f32
            tc.tile_pool(name="q8", bufs=1) as qpool,          # [128,2048] fp8
            tc.tile_pool(name="small", bufs=1) as small,
            tc.tile_pool(name="psum", bufs=8, space="PSUM") as psum,
            tc.tile_pool(name="dram", bufs=1, space="DRAM") as dram,
        ):
            # ---------------- DRAM scratch (flat, unit-major, fp8) -----------
            stage = [None] * 4
            image = [None] * 4
            for L in layers[1:]:
                li = L["li"]
                stage[li] = dram.tile([L["per_rank"] * CH_ELS], FP8,
                                      tag=f"stage{li}", name=f"stage{li}")
                image[li] = dram.tile([L["n_ch"] * CH_ELS], FP8,
                                      tag=f"image{li}", name=f"image{li}",
                                      addr_space="Shared")
            ar_in = [dram.tile([P, 1], F32, tag=f"ar_in{l}", name=f"ar_in{l}")
                     for l in range(2)]
            ar_out = [dram.tile([P, 1], F32, tag=f"ar_out{l}",
                                name=f"ar_out{l}", addr_space="Shared")
                      for l in range(2)]
            ar_in23 = dram.tile([P, 2], F32, tag="ar_in23", name="ar_in23")
            ar_out23 = dram.tile([P, 2], F32, tag="ar_out23",
                                 name="ar_out23", addr_space="Shared")

            RUN = 4
            # wsh offsets per layer (includes L0's shard for the abs pass)
            layer_jofs = {}
            jofs = 0
            for L in layers:
                layer_jofs[L["li"]] = jofs
                jofs += L["per_rank"]

            ABS_RUN = 2
            n_runs_total = (sum((L["per_rank"] + RUN - 1) // RUN
                                for L in layers[:2])
                            + sum((L["per_rank"] + ABS_RUN - 1) // ABS_RUN
                                  for L in layers[2:]))
            partials = small.tile([P, n_runs_total], F32, tag="partials",
                                  name="partials")
            mwb = [None] * 4
            swb = [None] * 4

            def _abs_pass_act(li, partial_col):
                """fp32 shard read + |.| accumulate on the ACT engine."""
                L = layers[li]
                pr, jofs = L["per_rank"], layer_jofs[li]
                nrun = 0
                for h in range(0, pr, RUN):
                    rl = min(RUN, pr - h)
                    off = (jofs + h) * CH_ELS
                    wrun = wrpool.tile([P, rl * OBW], F32, tag="wr",
                                       name=f"wrB{li}_{h}")
                    nc.scalar.dma_start(
                        wrun[:], wsh[off:off + rl * CH_ELS].rearrange(
                            "(p f) -> p f", p=P))
                    nc.scalar.activation(wrun[:], wrun[:], AF.Abs,
                                         bias=0.0, scale=1.0,
                                         accum_out=partials[:, partial_col + nrun:
                                                            partial_col + nrun + 1])
                    nrun += 1
                return nrun

            def _papr_ar(pm_src, ar_i, ar_o, ncols, name):
                """partition all-reduce -> DRAM -> AllReduce across ranks."""
                pall = small.tile([P, ncols], F32, tag=f"pall_{name}",
                                  name=f"pall_{name}")
                nc.gpsimd.partition_all_reduce(pall[:], pm_src,
                                               channels=P,
                                               reduce_op=bass_isa.ReduceOp.add)
                nc.sync.dma_start(ar_i[:], pall[:])
                nc.gpsimd.collective_compute(
                    "AllReduce", OP.add,
                    ins=[ar_i.opt()], outs=[ar_o.opt()],
                    replica_groups=[list(range(N_CORES))])

            def _swl_chain(li, pms_ap):
                """mean|W| and its reciprocal from the AllReduced sum (DVE)."""
                L = layers[li]
                mwl = small.tile([P, 1], F32, tag=f"mwb{li}", name=f"mwb{li}")
                nc.vector.tensor_scalar(mwl[:], pms_ap, 1.0 / L["numel"],
                                        float(EPS), OP.mult, OP.max)
                swl = small.tile([P, 1], F32, tag=f"swb{li}", name=f"swb{li}")
                nc.vector.reciprocal(swl[:], mwl[:])
                return mwl, swl

            def _quant_run_gpsimd(wrun, swl, rl, li, hh):
                """round(w*s) clip [-1,1] -> fp8, on GpSimd."""
                uu = upool.tile([P, rl * OBW], F32, tag="u", name=f"uq{li}_{hh}")
                nc.gpsimd.tensor_scalar(uu[:], wrun[:], swl[:], MAGIC,
                                        OP.mult, OP.add)
                nc.gpsimd.tensor_scalar(uu[:], uu[:], MAGIC, 1.0,
                                        OP.subtract, OP.min)
                qrun = qpool.tile([P, rl * OBW], FP8, tag="q8",
                                  name=f"qr{li}_{hh}")
                nc.gpsimd.tensor_scalar(qrun[:], uu[:], -1.0, None, OP.max)
                return qrun

            def _quant_layer_gpsimd(li, swl):
                """Cooperative quantize of rank's shard -> stage -> AllGather.
                DMA issues + math + stage writes all on the GpSimd queue
                (self-paced; keeps ACT/DVE/sync clear for the main pass)."""
                L = layers[li]
                pr, jofs = L["per_rank"], layer_jofs[li]
                for h in range(0, pr, RUN):
                    rl = min(RUN, pr - h)
                    off = (jofs + h) * CH_ELS
                    wrun = wrpool.tile([P, rl * OBW], F32, tag="wr",
                                       name=f"wrC{li}_{h}")
                    nc.gpsimd.dma_start(
                        wrun[:], wsh[off:off + rl * CH_ELS].rearrange(
                            "(p f) -> p f", p=P))
                    qrun = _quant_run_gpsimd(wrun, swl, rl, li, h)
                    soff = h * CH_ELS
                    nc.gpsimd.dma_start(
                        stage[li][soff:soff + rl * CH_ELS].rearrange(
                            "(p f) -> p f", p=P),
                        qrun[:])
                nc.gpsimd.collective_compute(
                    "AllGather", OP.bypass,
                    ins=[stage[li].opt()],
                    outs=[image[li].opt()],
                    replica_groups=[list(range(N_CORES))])

            # ======== P0: L0 abs (co-op) -> AR0 ========
            nr0 = _abs_pass_act(0, 0)
            pm0 = small.tile([P, 1], F32, tag="pm0", name="pm0")
            nc.vector.tensor_reduce(pm0[:], partials[:, 0:nr0],
                                    mybir.AxisListType.X, OP.add)
            _papr_ar(pm0[:], ar_in[0], ar_out[0], 1, "ar0")

            # ======== P1: input activation quant ========
            n_ic0 = L0["n_ic"]
            xqT_cur = []
            am0s = []
            for t in range(T):
                xt = ypool.tile([P, D0], F32, tag="y", name=f"xt{t}")
                nc.sync.dma_start(xt[:], xs[t * P:(t + 1) * P, :])
                am = small.tile([P, 1], F32, tag=f"am0_{t}", name=f"am0_{t}")
                nc.vector.tensor_reduce(am[:], xt[:], mybir.AxisListType.X,
                                        OP.max, apply_absolute_value=True)
                nc.vector.tensor_scalar(am[:], am[:], float(EPS), None, OP.max)
                sc = small.tile([P, 1], F32, tag=f"s0_{t}", name=f"s0_{t}")
                nc.vector.tensor_scalar(sc[:], am[:], 1.0 / 127.0, None, OP.mult)
                nc.vector.reciprocal(sc[:], sc[:])
                xq0 = xqnpool.tile([P, D0], BF16, tag="xqn", name=f"xq0_{t}")
                uu = upool.tile([P, D0], F32, tag="u", name=f"u0_{t}")
                nc.scalar.activation(uu[:], xt[:], AF.Copy, bias=MAGIC,
                                     scale=sc[:])
                nc.vector.tensor_scalar(xq0[:], uu[:], MAGIC, None, OP.subtract)
                xqT0 = xqT0pool.tile([P, n_ic0, P], BF16, tag="xqT0",
                                     name=f"xqT0_{t}")
                nc.sync.dma_start_transpose(xqT0[:], xq0[:])
                xqT_cur.append(xqT0)
                am0s.append(am)

            # ======== P2: L1 abs -> AR1 ========
            col = nr0
            cols = {1: col}
            col += _abs_pass_act(1, cols[1])
            pm1 = small.tile([P, 1], F32, tag="pm1", name="pm1")
            nc.vector.tensor_reduce(pm1[:], partials[:, cols[1]:col],
                                    mybir.AxisListType.X, OP.add)
            _papr_ar(pm1[:], ar_in[1], ar_out[1], 1, "ar1")

            # L2/L3 abs DMA issues on sync (reduced on DVE inside the L0 loop)
            wabs_runs = []  # (li, tile, rl, col)
            for li in (2, 3):
                L = layers[li]
                pr, jofs = L["per_rank"], layer_jofs[li]
                cols[li] = col
                for h in range(0, pr, ABS_RUN):
                    rl = min(ABS_RUN, pr - h)
                    off = (jofs + h) * CH_ELS
                    wrun = wabspool.tile([P, rl * OBW], F32, tag="wabs",
                                         name=f"wA{li}_{h}")
                    nc.sync.dma_start(
                        wrun[:], wsh[off:off + rl * CH_ELS].rearrange(
                            "(p f) -> p f", p=P))
                    wabs_runs.append((li, wrun, rl, col))
                    col += 1

            # ======== P3: L0 scale (waits AR0) ========
            pms0 = small.tile([P, 1], F32, tag="pms0", name="pms0")
            nc.gpsimd.dma_start(pms0[:], ar_out[0][:])
            mwb[0], swb[0] = _swl_chain(0, pms0[:])
            c_cur = []
            for t in range(T):
                c0 = small.tile([P, 1], F32, tag=f"c0_{t}", name=f"c0_{t}")
                nc.vector.scalar_tensor_tensor(c0[:], am0s[t][:], 1.0 / 127.0,
                                               mwb[0][:], OP.mult, OP.mult)
                c_cur.append(c0)

            pms1 = small.tile([P, 1], F32, tag="pms1", name="pms1")
            nc.gpsimd.dma_start(pms1[:], ar_out[1][:])

            # ======== P4: main pass L0 with fused local quant ========
            # w0f chunk DMAs issued from the Tensor queue: self-paced against
            # the DVE quant that frees the ring slots.
            Q0RUN = min(RUN, n_ic0)
            n_q0runs = n_ic0 // Q0RUN
            abs_iter = iter(wabs_runs)
            abs_done = []

            def _emit_l0(ob, do_swl1):
                wp8 = wp0pool.tile([P, n_ic0, OBW], FP8, tag="wp0",
                                   name=f"wp0_{ob}")
                for r in range(n_q0runs):
                    off = (ob * n_ic0 + r * Q0RUN) * CH_ELS
                    w0r = w0pool.tile([P, Q0RUN * OBW], F32, tag="w0",
                                      name=f"w0r_{ob}_{r}")
                    nc.scalar.dma_start(
                        w0r[:], w0f[off:off + Q0RUN * CH_ELS].rearrange(
                            "(p f) -> p f", p=P))
                    uu = upool.tile([P, Q0RUN * OBW], F32, tag="u",
                                    name=f"u0q_{ob}_{r}")
                    nc.vector.tensor_scalar(uu[:], w0r[:], swb[0][:], MAGIC,
                                            OP.mult, OP.add)
                    nc.vector.tensor_scalar(uu[:], uu[:], MAGIC, 1.0,
                                            OP.subtract, OP.min)
                    nc.vector.tensor_scalar(
                        wp8[:, r * Q0RUN:(r + 1) * Q0RUN, :]
                        .rearrange("p a b -> p (a b)"),
                        uu[:], -1.0, None, OP.max)
                # interleave two L2/L3 abs reduces per ob (keeps DVE fed)
                for _ in range(2):
                    nxt = next(abs_iter, None)
                    if nxt is not None:
                        li_, wr_, rl_, c_ = nxt
                        nc.vector.tensor_reduce(partials[:, c_:c_ + 1], wr_[:],
                                                mybir.AxisListType.X, OP.add,
                                                apply_absolute_value=True)
                        abs_done.append(nxt)
                if do_swl1:  # placed mid-loop: AR1 has landed by now
                    mwb[1], swb[1] = _swl_chain(1, pms1[:])
                return wp8

            ys = [ypool.tile([P, L0["dreal"]], F32, tag="y", name=f"y0_{t}")
                  for t in range(T)]
            bns = [small.tile([P, L0["n_ob"] * 6], F32, tag=f"bn{t}",
                              name=f"bn0_{t}") for t in range(T)]
            for ob in range(L0["n_ob"]):
                wp8 = _emit_l0(ob, do_swl1=(ob == min(3, L0["n_ob"] - 1)))
                ps = [psum.tile([P, OBW], F32, tag="ps",
                                name=f"ps0_{ob}_{t}") for t in range(T)]
                for t in range(T):
                    for c in range(n_ic0):
                        nc.tensor.matmul(ps[t][:], xqT_cur[t][:, c, :],
                                         wp8[:, c, :],
                                         start=(c == 0), stop=(c == n_ic0 - 1))
                for t in range(T):
                    dst = ys[t][:, ob * OBW:(ob + 1) * OBW]
                    nc.scalar.activation(dst, ps[t][:], AF.Copy,
                                         bias=0.0, scale=c_cur[t][:])

            # drain any remaining abs reduces
            for nxt in abs_iter:
                li_, wr_, rl_, c_ = nxt
                nc.vector.tensor_reduce(partials[:, c_:c_ + 1], wr_[:],
                                        mybir.AxisListType.X, OP.add,
                                        apply_absolute_value=True)

            # ======== P5: L1 quant + AllGather (GpSimd) ========
            _quant_layer_gpsimd(1, swb[1])

            # ======== P6: AR23 ========
            pm23 = small.tile([P, 2], F32, tag="pm23", name="pm23")
            for i, li in enumerate((2, 3)):
                hi = col if li == 3 else cols[3]
                nc.vector.tensor_reduce(pm23[:, i:i + 1],
                                        partials[:, cols[li]:hi],
                                        mybir.AxisListType.X, OP.add)
            _papr_ar(pm23[:], ar_in23, ar_out23, 2, "ar23")
            pms23 = small.tile([P, 2], F32, tag="pms23", name="pms23")
            nc.gpsimd.dma_start(pms23[:], ar_out23[:])

            # ======== P7: L0 tail + remaining quant + main L1-3 ========
            def _tail(li, ys, bns, c_list):
                """LN+SiLU (fused, hw table) + act quant + transpose."""
                L = layers[li]
                dout = L["dout"]
                n_ic_next = layers[li + 1]["n_ic"]
                QW = min(2048, dout)
                xqT_next, c_next = [], []
                for t in range(T):
                    if li == 0:  # bn_stats deferred (DVE was quantizing)
                        for ob in range(L["n_ob"]):
                            nc.vector.bn_stats(
                                bns[t][:, ob * 6:(ob + 1) * 6],
                                ys[t][:, ob * OBW:(ob + 1) * OBW])
                    mv = small.tile([P, 2], F32, tag="mv", name=f"mv{li}_{t}")
                    nc.vector.bn_aggr(mv[:], bns[t][:])
                    v = small.tile([P, 1], F32, tag="vvar", name=f"v{li}_{t}")
                    nc.vector.tensor_scalar(v[:], mv[:, 1:2], float(EPS), None,
                                            OP.add)
                    istd = _rsqrt_newton(nc, small, v[:])
                    nmi = small.tile([P, 1], F32, tag="nmi", name=f"nmi{li}_{t}")
                    nc.vector.scalar_tensor_tensor(nmi[:], mv[:, 0:1], -1.0,
                                                   istd[:], OP.mult, OP.mult)
                    nc.scalar.activation(ys[t][:], ys[t][:], AF.Silu,
                                         bias=nmi[:], scale=istd[:])
                    am = small.tile([P, 1], F32, tag="amn", name=f"am{li}_{t}")
                    nc.vector.tensor_reduce(am[:], ys[t][:],
                                            mybir.AxisListType.X, OP.max,
                                            apply_absolute_value=True)
                    nc.vector.tensor_scalar(am[:], am[:], float(EPS), None,
                                            OP.max)
                    sc = small.tile([P, 1], F32, tag="scn", name=f"sc{li}_{t}")
                    nc.vector.tensor_scalar(sc[:], am[:], 1.0 / 127.0, None,
                                            OP.mult)
                    nc.vector.reciprocal(sc[:], sc[:])
                    cn = small.tile([P, 1], F32, tag=f"c{li + 1}_{t}",
                                    name=f"c{li + 1}_{t}")
                    nc.vector.scalar_tensor_tensor(cn[:], am[:], 1.0 / 127.0,
                                                   mwb[li + 1][:],
                                                   OP.mult, OP.mult)
                    c_next.append(cn)
                    xqn = xqnpool.tile([P, dout], BF16, tag="xqn",
                                       name=f"xqn{li}_{t}")
                    xT = xqTpool.tile([P, n_ic_next, P], BF16, tag="xqT",
                                      name=f"xT{li}_{t}")
                    icq = QW // P
                    for ch in range(dout // QW):
                        uu = upool.tile([P, QW], F32, tag="u",
                                        name=f"ur{li}_{t}_{ch}")
                        nc.scalar.activation(uu[:],
                                             ys[t][:, ch * QW:(ch + 1) * QW],
                                             AF.Copy, bias=MAGIC, scale=sc[:])
                        nc.vector.tensor_scalar(xqn[:, ch * QW:(ch + 1) * QW],
                                                uu[:], MAGIC, None, OP.subtract)
                        # transpose each finished half immediately
                        nc.sync.dma_start_transpose(
                            xT[:, ch * icq:(ch + 1) * icq, :],
                            xqn[:, ch * QW:(ch + 1) * QW])
                    xqT_next.append(xT)
                return xqT_next, c_next

            xqT_cur, c_cur = _tail(0, ys, bns, c_cur)

            # L1 scale for next dequant was computed in P4; L2/L3 swl now.
            # (AR23 lands well before the DVE reaches this point.)
            mwb[2], swb[2] = _swl_chain(2, pms23[:, 0:1])
            mwb[3], swb[3] = _swl_chain(3, pms23[:, 1:2])
            _quant_layer_gpsimd(2, swb[2])
            _quant_layer_gpsimd(3, swb[3])

            for L in layers[1:]:
                li, n_ic, n_ob = L["li"], L["n_ic"], L["n_ob"]
                panel_ic, n_panels = L["panel_ic"], L["n_panels"]
                dout, dreal = L["dout"], L["dreal"]
                is_last = (li == 3)

                ys = [ypool.tile([P, dreal], F32, tag="y", name=f"y{li}_{t}")
                      for t in range(T)]
                bns = [small.tile([P, n_ob * 6], F32, tag=f"bn{t}",
                                  name=f"bn{li}_{t}")
                       for t in range(T)] if not is_last else None

                for ob in range(n_ob):
                    ow = L["ob_w"][ob]
                    ps = [psum.tile([P, OBW], F32, tag="ps",
                                    name=f"ps{li}_{ob}_{t}") for t in range(T)]
                    for panel in range(n_panels):
                        wp = wpool.tile([P, panel_ic, OBW], FP8, tag="wp",
                                        name=f"wp{li}_{ob}_{panel}")
                        g0 = (ob * n_ic + panel * panel_ic)
                        uoff = g0 * CH_ELS
                        nc.sync.dma_start(
                            wp[:], image[li][uoff:uoff + panel_ic * CH_ELS]
                            .rearrange("(p c f) -> p c f", p=P, c=panel_ic))
                        for t in range(T):
                            for cc in range(panel_ic):
                                c = panel * panel_ic + cc
                                nc.tensor.matmul(
                                    ps[t][:], xqT_cur[t][:, c, :],
                                    wp[:, cc, :],
                                    start=(c == 0), stop=(c == n_ic - 1))
                    for t in range(T):
                        dst = ys[t][:, ob * OBW:ob * OBW + ow]
                        if not is_last:
                            nc.scalar.activation(dst, ps[t][:, :ow], AF.Copy,
                                                 bias=0.0, scale=c_cur[t][:])
                            nc.vector.bn_stats(bns[t][:, ob * 6:(ob + 1) * 6],
                                               dst)
                        else:
                            nc.scalar.activation(dst, ps[t][:, :ow], AF.Sigmoid,
                                                 bias=0.0, scale=c_cur[t][:])

                if is_last:
                    for t in range(T):
                        mzt = upool.tile([P, OBINS], F32, tag="u",
                                           name=f"mzt{t}")
                        nc.vector.tensor_scalar(mzt[:], ys[t][:, 0:OBINS],
                                                float(OBINS - 1), 1.0,
                                                OP.mult, OP.add)
                        nc.scalar.dma_start(mz_out[t * P:(t + 1) * P, :], mzt[:])
                        iit = upool.tile([P, OBINS], F32, tag="u",
                                           name=f"iit{t}")
                        nc.vector.tensor_scalar(iit[:], ys[t][:, OBINS:2 * OBINS],
                                                100.0, None, OP.mult)
                        nc.scalar.dma_start(ii_out[t * P:(t + 1) * P, :], iit[:])
                    continue

                xqT_cur, c_cur = _tail(li, ys, bns, c_cur)

    nc.compile()
    return nc


def prepare_inputs(cfg, x, W0, W1, W2, W3):
    """Host-side sharding: per-core input maps. Weight chunks are shipped
    unit-major: unit u = (layer, ob, panel) is a [128, panel_ic*512] block,
    rows = partitions, contiguous per row; chunk cc of the unit holds
    W_l[ob*512+o, (panel*panel_ic+cc)*128+p] at [p, cc*512+o] (i.e. W^T).
    W0 additionally ships as a full ob-major image (same for all cores) for
    the local layer-0 quantize."""
    layers, total_ch, per_rank, T = _plan(cfg)
    b_core = T * P
    Ws = [np.asarray(W0), np.asarray(W1), np.asarray(W2), np.asarray(W3)]
    WTs = []
    for L, W in zip(layers, Ws):
        WT = np.zeros((L["din"], L["dout"]), dtype=np.float32)
        WT[:, :L["dreal"]] = W.T
        WTs.append(WT)

    shards = [np.empty(per_rank * CH_ELS, dtype=np.float32)
              for _ in range(N_CORES)]
    for L in layers:
        li, pr = L["li"], L["per_rank"]
        n_ic, panel_ic = L["n_ic"], L["panel_ic"]
        WT = WTs[li]
        for r in range(N_CORES):
            g0 = r * pr
            dst = shards[r]
            for j in range(0, pr, panel_ic):
                g = g0 + j
                ob, ic0 = divmod(g, n_ic)
                assert ic0 % panel_ic == 0
                blk = WT[ic0 * P:(ic0 + panel_ic) * P,
                         ob * OBW:(ob + 1) * OBW]          # [panel_ic*128, 512]
                blk = blk.reshape(panel_ic, P, OBW).transpose(1, 0, 2)
                off = (L["ch_base"] // N_CORES + j) * CH_ELS
                dst[off:off + panel_ic * CH_ELS] = blk.reshape(-1)

    # full W0 image, ob-major, grouped per quant run of Q0RUN chunks
    L0 = layers[0]
    n_ic0, n_ob0 = L0["n_ic"], L0["n_ob"]
    q0run = min(4, n_ic0)
    WT0 = WTs[0]
    w0f = np.empty(L0["n_ch"] * CH_ELS, dtype=np.float32)
    pos = 0
    for ob in range(n_ob0):
        for r0 in range(0, n_ic0, q0run):
            blk = WT0[r0 * P:(r0 + q0run) * P, ob * OBW:(ob + 1) * OBW]
            blk = blk.reshape(q0run, P, OBW).transpose(1, 0, 2)
            w0f[pos:pos + q0run * CH_ELS] = blk.reshape(-1)
            pos += q0run * CH_ELS

    x = np.asarray(x, dtype=np.float32)
    in_maps = []
    for r in range(N_CORES):
        in_maps.append(dict(
            xs=np.ascontiguousarray(x[r * b_core:(r + 1) * b_core]),
            wsh=shards[r],
            w0f=w0f,
        ))
    return in_maps


_NC_CACHE = {}


def _get_nc(cfg_key):
    if cfg_key not in _NC_CACHE:
        _NC_CACHE[cfg_key] = build(dict(cfg_key))
    return _NC_CACHE[cfg_key]


def run(cfg, x, W0, W1, W2, W3, trace=False):
    layers, total_ch, per_rank, T = _plan(cfg)
    b_core = T * P
    nc = _get_nc(tuple(sorted(cfg.items())))
    in_maps = prepare_inputs(cfg, x, W0, W1, W2, W3)
    res = run_bass_kernel_spmd(nc, in_maps, core_ids=list(range(N_CORES)),
                               trace=trace)
    mz = np.concatenate([res.results[r]["mz"] for r in range(N_CORES)], axis=0)
    ii = np.concatenate([res.results[r]["ii"] for r in range(N_CORES)], axis=0)
    return (mz, ii), res


def kernel(x, W0, W1, W2, W3, g0, b0, g1, b1, g2, b2):
    """Full-input entry point. g/b are identity (ones/zeros) in this problem's
    setup; LayerNorm affine is a no-op and is validated here."""
    for g in (g0, g1, g2):
        assert np.allclose(np.asarray(g), 1.0), "non-identity LN gain unsupported"
    for b in (b0, b1, b2):
        assert np.allclose(np.asarray(b), 0.0), "non-zero LN bias unsupported"
    (mz, ii), _ = run(FULL_CFG, x, W0, W1, W2, W3, trace=False)
    return (mz, ii)


# revision 15
# speedup vs baseline: 1.8289x; 1.8289x over previous
"""BitNet decoder MLP on 8 Trainium2 NeuronCores (Bass/Tile).

Strategy: data-parallel over batch (512 rows/core).

Weights are ternary-quantized on device. Layer 0 is quantized locally on every
core from a full ob-major fp32 copy (its abs-mean is still computed
cooperatively + AllReduce), so L0 matmuls start as soon as the scale lands —
no AllGather on the critical path. Layers 1-3 are quantized cooperatively
(1/8 per core) into an fp8e4 image and AllGathered; collectives have a large
fixed cost and share one serial queue, so there is exactly one gather per
layer, ordered AR0 -> AR1 -> AG_L1 -> AR23 -> AG_L2 -> AG_L3.

The quantized image is fp8e4 (E4M3): ternary {-1,0,1} is exact in fp8, and
the tensor engine accepts mixed-dtype matmuls (bf16 stationary activations x
fp8 moving weights) at full rate, so the image is half the bytes of bf16 with
identical matmul time. All matmul arithmetic is exact: activations are
int8-valued bf16, weights {-1,0,1} fp8, accumulation fp32 in PSUM.

Engine budget: PE does matmuls (the 605us floor); ACT does abs passes, PSUM
eviction (dequant scale folded in) and the fused LN+SiLU (hw Silu table);
DVE does L0 quant, stats, act-quant; GpSimd does L1-3 quant + collectives;
transposes ride the DMA crossbar. Rounding via the fp32 magic-number trick
(round-half-even, matches jnp.round).
"""

import numpy as np

import concourse.bass as bass
import concourse.bass_isa as bass_isa
import concourse.mybir as mybir
import concourse.tile as tile
from concourse import bacc
from concourse.bass_utils import run_bass_kernel_spmd

F32 = mybir.dt.float32
BF16 = mybir.dt.bfloat16
FP8 = mybir.dt.float8e4
AF = mybir.ActivationFunctionType
OP = mybir.AluOpType

N_CORES = 8
P = 128
OBW = 512            # output block width (one PSUM bank of fp32)
CH_ELS = P * OBW     # elements per weight chunk
MAGIC = 12582912.0   # 1.5 * 2**23: fp32 round-to-nearest-even trick
EPS = 1e-5

FULL_CFG = dict(B=4096, D0=1024, H=4096, OBINS=1000)


def _plan(cfg):
    """Static per-layer plan."""
    B, D0, H, OBINS = cfg["B"], cfg["D0"], cfg["H"], cfg["OBINS"]
    o3_real = 2 * OBINS
    o3_pad = ((o3_real + OBW - 1) // OBW) * OBW
    dims = [
        dict(din=D0, dout=H, dreal=H),
        dict(din=H, dout=H, dreal=H),
        dict(din=H, dout=H, dreal=H),
        dict(din=H, dout=o3_pad, dreal=o3_real),
    ]
    numels = [H * D0, H * H, H * H, o3_real * H]  # real numels for mean|W|
    layers = []
    ch_base = 0
    for li, d in enumerate(dims):
        n_ic = d["din"] // P
        n_ob = d["dout"] // OBW
        n_ch = n_ob * n_ic
        assert n_ch % N_CORES == 0, (li, n_ch)
        panel_ic = min(8, n_ic, max(1, n_ch // N_CORES))
        assert n_ic % panel_ic == 0 and (n_ch // N_CORES) % panel_ic == 0
        n_panels = n_ic // panel_ic
        layers.append(dict(
            li=li, din=d["din"], dout=d["dout"], dreal=d["dreal"],
            n_ic=n_ic, n_ob=n_ob, n_ch=n_ch, per_rank=n_ch // N_CORES,
            panel_ic=panel_ic, n_panels=n_panels,
            numel=numels[li], ch_base=ch_base,
            ob_w=[min(OBW, d["dreal"] - ob * OBW) for ob in range(n_ob)],
        ))
        ch_base += n_ch
    total_ch = ch_base
    per_rank = total_ch // N_CORES
    b_core = B // N_CORES
    assert b_core % P == 0
    return layers, total_ch, per_rank, b_core // P


def _rsqrt_newton(nc, pool, v, n_iter=3):
    """istd = 1/sqrt(v) for v [128,1] fp32 (v > 0), pure-DVE Newton iteration.

    seed_bits = 0x5f370000 - bits(v)/2 computed in fp32 on aligned int32 views;
    the fp32 mantissa noise on the >2^24 intermediate is irrelevant for a seed.
    """
    seed = pool.tile([P, 1], F32, tag="rs_seed", name="rs_seed")
    seed_i32 = seed[:].bitcast(mybir.dt.int32)
    v_i32 = v.bitcast(mybir.dt.int32)
    nc.vector.tensor_scalar(seed_i32[:], v_i32[:], -0.5,
                            float(0x5F370000), OP.mult, OP.add)
    y = seed
    t1 = pool.tile([P, 1], F32, tag="rs_t1", name="rs_t1")
    t2 = pool.tile([P, 1], F32, tag="rs_t2", name="rs_t2")
    for _ in range(n_iter):
        nc.vector.tensor_tensor(t1[:], y[:], y[:], OP.mult)
        nc.vector.tensor_tensor(t2[:], t1[:], v, OP.mult)
        nc.vector.tensor_scalar(t1[:], t2[:], -0.5, 1.5, OP.mult, OP.add)
        nc.vector.tensor_tensor(y[:], y[:], t1[:], OP.mult)
    return y


def build(cfg):
    layers, total_ch, per_rank, T = _plan(cfg)
    nc = bacc.Bacc("TRN2", target_bir_lowering=False, debug=False,
                   num_devices=N_CORES)

    D0, OBINS = cfg["D0"], cfg["OBINS"]
    b_core = T * P
    L0 = layers[0]
    n_ch0 = L0["n_ch"]

    xs = nc.dram_tensor("xs", [b_core, D0], F32, kind="ExternalInput")
    # rank's weight chunks (all layers), unit-major flat fp32
    wsh = nc.dram_tensor("wsh", [per_rank * CH_ELS], F32, kind="ExternalInput")
    # full W0, ob-major unit layout (same copy on every core)
    w0f = nc.dram_tensor("w0f", [n_ch0 * CH_ELS], F32, kind="ExternalInput")
    mz_out = nc.dram_tensor("mz", [b_core, OBINS], F32, kind="ExternalOutput")
    ii_out = nc.dram_tensor("ii", [b_core, OBINS], F32, kind="ExternalOutput")

    with tile.TileContext(nc) as tc:
        with (
            tc.tile_pool(name="ybig", bufs=4) as ypool,        # 16KB/partition f32
            tc.tile_pool(name="wr", bufs=3) as wrpool,         # prep fp32 runs 8KB
            tc.tile_pool(name="w0", bufs=2) as w0pool,         # L0 quant src 8KB
            tc.tile_pool(name="wabs", bufs=2) as wabspool,     # L2/L3 abs runs 8KB
            tc.tile_pool(name="xqT", bufs=4) as xqTpool,       # [128,32,128] bf16
            tc.tile_pool(name="xqT0", bufs=4) as xqT0pool,     # [128,n_ic0,128] bf16
            tc.tile_pool(name="wp", bufs=2) as wpool,          # [128,16,512] fp8
            tc.tile_pool(name="wp0", bufs=2) as wp0pool,       # [128,n_ic0,512] fp8
            tc.tile_pool(name="xqn", bufs=2) as xqnpool,       # 8KB/partition bf16
            tc.tile_pool(name="u", bufs=2) as upool,           # [128,2048] f32
            tc.tile_pool(name="q8", bufs=1) as qpool,          # [128,2048] fp8
            tc.tile_pool(name="small", bufs=1) as small,
            tc.tile_pool(name="psum", bufs=8, space="PSUM") as psum,
            tc.tile_pool(name="dram", bufs=1, space="DRAM") as dram,
        ):
            # ---------------- DRAM scratch (flat, unit-major, fp8) -----------
            stage = [None] * 4
            image = [None] * 4
            for L in layers[1:]:
                li = L["li"]
                stage[li] = dram.tile([L["per_rank"] * CH_ELS], FP8,
                                      tag=f"stage{li}", name=f"stage{li}")
                image[li] = dram.tile([L["n_ch"] * CH_ELS], FP8,
                                      tag=f"image{li}", name=f"image{li}",
                                      addr_space="Shared")
            ar_in = [dram.tile([P, 1], F32, tag=f"ar_in{l}", name=f"ar_in{l}")
                     for l in range(2)]
            ar_out = [dram.tile([P, 1], F32, tag=f"ar_out{l}",
                                name=f"ar_out{l}", addr_space="Shared")
                      for l in range(2)]
            ar_in23 = dram.tile([P, 2], F32, tag="ar_in23", name="ar_in23")
            ar_out23 = dram.tile([P, 2], F32, tag="ar_out23",
                                 name="ar_out23", addr_space="Shared")

            RUN = 4
            # wsh offsets per layer (includes L0's shard for the abs pass)
            layer_jofs = {}
            jofs = 0
            for L in layers:
                layer_jofs[L["li"]] = jofs
                jofs += L["per_rank"]

            ABS_RUN = 2
            n_runs_total = (sum((L["per_rank"] + RUN - 1) // RUN
                                for L in layers[:2])
                            + sum((L["per_rank"] + ABS_RUN - 1) // ABS_RUN
                                  for L in layers[2:]))
            partials = small.tile([P, n_runs_total], F32, tag="partials",
                                  name="partials")
            mwb = [None] * 4
            swb = [None] * 4

            def _abs_pass_act(li, partial_col):
                """fp32 shard read + |.| accumulate on the ACT engine."""
                L = layers[li]
                pr, jofs = L["per_rank"], layer_jofs[li]
                nrun = 0
                for h in range(0, pr, RUN):
                    rl = min(RUN, pr - h)
                    off = (jofs + h) * CH_ELS
                    wrun = wrpool.tile([P, rl * OBW], F32, tag="wr",
                                       name=f"wrB{li}_{h}")
                    nc.scalar.dma_start(
                        wrun[:], wsh[off:off + rl * CH_ELS].rearrange(
                            "(p f) -> p f", p=P))
                    nc.scalar.activation(wrun[:], wrun[:], AF.Abs,
                                         bias=0.0, scale=1.0,
                                         accum_out=partials[:, partial_col + nrun:
                                                            partial_col + nrun + 1])
                    nrun += 1
                return nrun

            def _papr_ar(pm_src, ar_i, ar_o, ncols, name):
                """partition all-reduce -> DRAM -> AllReduce across ranks."""
                pall = small.tile([P, ncols], F32, tag=f"pall_{name}",
                                  name=f"pall_{name}")
                nc.gpsimd.partition_all_reduce(pall[:], pm_src,
                                               channels=P,
                                               reduce_op=bass_isa.ReduceOp.add)
                nc.gpsimd.dma_start(ar_i[:], pall[:])
                nc.gpsimd.collective_compute(
                    "AllReduce", OP.add,
                    ins=[ar_i.opt()], outs=[ar_o.opt()],
                    replica_groups=[list(range(N_CORES))])

            def _swl_chain(li, pms_ap):
                """mean|W| and its reciprocal from the AllReduced sum (DVE)."""
                L = layers[li]
                mwl = small.tile([P, 1], F32, tag=f"mwb{li}", name=f"mwb{li}")
                nc.vector.tensor_scalar(mwl[:], pms_ap, 1.0 / L["numel"],
                                        float(EPS), OP.mult, OP.max)
                swl = small.tile([P, 1], F32, tag=f"swb{li}", name=f"swb{li}")
                nc.vector.reciprocal(swl[:], mwl[:])
                return mwl, swl

            def _emit_quant_run(li, h, swl):
                """One shard run: GpSimd DMA in, DVE quant math, GpSimd
                stage write. round(w*s) via magic, clip [-1,1], fp8 out."""
                L = layers[li]
                pr, jofs = L["per_rank"], layer_jofs[li]
                rl = min(RUN, pr - h)
                off = (jofs + h) * CH_ELS
                wrun = wrpool.tile([P, rl * OBW], F32, tag="wr",
                                   name=f"wrC{li}_{h}")
                nc.gpsimd.dma_start(
                    wrun[:], wsh[off:off + rl * CH_ELS].rearrange(
                        "(p f) -> p f", p=P))
                uu = upool.tile([P, rl * OBW], F32, tag="u", name=f"uq{li}_{h}")
                nc.vector.tensor_scalar(uu[:], wrun[:], swl[:], MAGIC,
                                        OP.mult, OP.add)
                nc.vector.tensor_scalar(uu[:], uu[:], MAGIC, 1.0,
                                        OP.subtract, OP.min)
                qrun = qpool.tile([P, rl * OBW], FP8, tag="q8",
                                  name=f"qr{li}_{h}")
                nc.vector.tensor_scalar(qrun[:], uu[:], -1.0, None, OP.max)
                soff = h * CH_ELS
                nc.gpsimd.dma_start(
                    stage[li][soff:soff + rl * CH_ELS].rearrange(
                        "(p f) -> p f", p=P),
                    qrun[:])

            def _gather_layer(li):
                nc.gpsimd.collective_compute(
                    "AllGather", OP.bypass,
                    ins=[stage[li].opt()],
                    outs=[image[li].opt()],
                    replica_groups=[list(range(N_CORES))])

            def _quant_layer_coop(li, swl):
                L = layers[li]
                for h in range(0, L["per_rank"], RUN):
                    _emit_quant_run(li, h, swl)
                _gather_layer(li)

            # ======== Barrier: absorb NEFF-start skew across ranks so the
            # first real AllReduce has low latency ========
            bar_in = dram.tile([1, 1], F32, tag="bar_in", name="bar_in")
            bar_out = dram.tile([1, 1], F32, tag="bar_out", name="bar_out",
                                addr_space="Shared")
            bar_sb = small.tile([1, 1], F32, tag="bar_sb", name="bar_sb")
            nc.vector.memset(bar_sb[:], 0.0)
            nc.gpsimd.dma_start(bar_in[:], bar_sb[:])
            nc.gpsimd.collective_compute(
                "AllReduce", OP.add,
                ins=[bar_in.opt()], outs=[bar_out.opt()],
                replica_groups=[list(range(N_CORES))])

            # ======== P0: L0 abs (co-op) -> AR0 ========
            nr0 = _abs_pass_act(0, 0)
            pm0 = small.tile([P, 1], F32, tag="pm0", name="pm0")
            nc.vector.tensor_reduce(pm0[:], partials[:, 0:nr0],
                                    mybir.AxisListType.X, OP.add)
            _papr_ar(pm0[:], ar_in[0], ar_out[0], 1, "ar0")

            # ======== P1: input activation quant ========
            n_ic0 = L0["n_ic"]
            xqT_cur = []
            am0s = []
            for t in range(T):
                xt = ypool.tile([P, D0], F32, tag="y", name=f"xt{t}")
                nc.sync.dma_start(xt[:], xs[t * P:(t + 1) * P, :])
                am = small.tile([P, 1], F32, tag=f"am0_{t}", name=f"am0_{t}")
                nc.vector.tensor_reduce(am[:], xt[:], mybir.AxisListType.X,
                                        OP.max, apply_absolute_value=True)
                nc.vector.tensor_scalar(am[:], am[:], float(EPS), None, OP.max)
                sc = small.tile([P, 1], F32, tag=f"s0_{t}", name=f"s0_{t}")
                nc.vector.tensor_scalar(sc[:], am[:], 1.0 / 127.0, None, OP.mult)
                nc.vector.reciprocal(sc[:], sc[:])
                xq0 = xqnpool.tile([P, D0], BF16, tag="xqn", name=f"xq0_{t}")
                uu = upool.tile([P, D0], F32, tag="u", name=f"u0_{t}")
                nc.scalar.activation(uu[:], xt[:], AF.Copy, bias=MAGIC,
                                     scale=sc[:])
                nc.vector.tensor_scalar(xq0[:], uu[:], MAGIC, None, OP.subtract)
                xqT0 = xqT0pool.tile([P, n_ic0, P], BF16, tag="xqT0",
                                     name=f"xqT0_{t}")
                nc.sync.dma_start_transpose(xqT0[:], xq0[:])
                xqT_cur.append(xqT0)
                am0s.append(am)

            # ======== P2: L1 abs -> AR1 ========
            col = nr0
            cols = {1: col}
            col += _abs_pass_act(1, cols[1])
            pm1 = small.tile([P, 1], F32, tag="pm1", name="pm1")
            nc.vector.tensor_reduce(pm1[:], partials[:, cols[1]:col],
                                    mybir.AxisListType.X, OP.add)
            _papr_ar(pm1[:], ar_in[1], ar_out[1], 1, "ar1")

            # L2/L3 abs DMA issues on sync (reduced on DVE inside the L0 loop)
            wabs_runs = []  # (li, tile, rl, col)
            for li in (2, 3):
                L = layers[li]
                pr, jofs = L["per_rank"], layer_jofs[li]
                cols[li] = col
                for h in range(0, pr, ABS_RUN):
                    rl = min(ABS_RUN, pr - h)
                    off = (jofs + h) * CH_ELS
                    wrun = wabspool.tile([P, rl * OBW], F32, tag="wabs",
                                         name=f"wA{li}_{h}")
                    nc.sync.dma_start(
                        wrun[:], wsh[off:off + rl * CH_ELS].rearrange(
                            "(p f) -> p f", p=P))
                    wabs_runs.append((li, wrun, rl, col))
                    col += 1

            # ======== P3: L0 scale (waits AR0) ========
            pms0 = small.tile([P, 1], F32, tag="pms0", name="pms0")
            nc.gpsimd.dma_start(pms0[:], ar_out[0][:])
            mwb[0], swb[0] = _swl_chain(0, pms0[:])
            c_cur = []
            for t in range(T):
                c0 = small.tile([P, 1], F32, tag=f"c0_{t}", name=f"c0_{t}")
                nc.vector.scalar_tensor_tensor(c0[:], am0s[t][:], 1.0 / 127.0,
                                               mwb[0][:], OP.mult, OP.mult)
                c_cur.append(c0)

            pms1 = small.tile([P, 1], F32, tag="pms1", name="pms1")
            nc.gpsimd.dma_start(pms1[:], ar_out[1][:])

            # ======== P4: main pass L0 with fused local quant ========
            # w0f chunk DMAs issued from the Tensor queue: self-paced against
            # the DVE quant that frees the ring slots.
            Q0RUN = min(RUN, n_ic0)
            n_q0runs = n_ic0 // Q0RUN
            abs_iter = iter(wabs_runs)
            abs_done = []
            l1_run_iter = iter(range(0, layers[1]["per_rank"], RUN))

            def _emit_l0(ob, do_swl1):
                wp8 = wp0pool.tile([P, n_ic0, OBW], FP8, tag="wp0",
                                   name=f"wp0_{ob}")
                for r in range(n_q0runs):
                    off = (ob * n_ic0 + r * Q0RUN) * CH_ELS
                    w0r = w0pool.tile([P, Q0RUN * OBW], F32, tag="w0",
                                      name=f"w0r_{ob}_{r}")
                    nc.scalar.dma_start(
                        w0r[:], w0f[off:off + Q0RUN * CH_ELS].rearrange(
                            "(p f) -> p f", p=P))
                    uu = upool.tile([P, Q0RUN * OBW], F32, tag="u",
                                    name=f"u0q_{ob}_{r}")
                    nc.scalar.activation(uu[:], w0r[:], AF.Copy, bias=MAGIC,
                                         scale=swb[0][:])
                    nc.vector.tensor_scalar(uu[:], uu[:], MAGIC, 1.0,
                                            OP.subtract, OP.min)
                    nc.vector.tensor_scalar(
                        wp8[:, r * Q0RUN:(r + 1) * Q0RUN, :]
                        .rearrange("p a b -> p (a b)"),
                        uu[:], -1.0, None, OP.max)
                # interleave two L2/L3 abs reduces per ob (keeps DVE fed)
                for _ in range(2):
                    nxt = next(abs_iter, None)
                    if nxt is not None:
                        li_, wr_, rl_, c_ = nxt
                        nc.vector.tensor_reduce(partials[:, c_:c_ + 1], wr_[:],
                                                mybir.AxisListType.X, OP.add,
                                                apply_absolute_value=True)
                        abs_done.append(nxt)
                if do_swl1:  # placed mid-loop: AR1 has landed by now
                    mwb[1], swb[1] = _swl_chain(1, pms1[:])
                if swb[1] is not None:
                    for _ in range(2):
                        h1 = next(l1_run_iter, None)
                        if h1 is not None:
                            _emit_quant_run(1, h1, swb[1])
                return wp8

            ys = [ypool.tile([P, L0["dreal"]], F32, tag="y", name=f"y0_{t}")
                  for t in range(T)]
            bns = [small.tile([P, L0["n_ob"] * 6], F32, tag=f"bn{t}",
                              name=f"bn0_{t}") for t in range(T)]
            for ob in range(L0["n_ob"]):
                wp8 = _emit_l0(ob, do_swl1=(ob == min(3, L0["n_ob"] - 1)))
                ps = [psum.tile([P, OBW], F32, tag="ps",
                                name=f"ps0_{ob}_{t}") for t in range(T)]
                for t in range(T):
                    for c in range(n_ic0):
                        nc.tensor.matmul(ps[t][:], xqT_cur[t][:, c, :],
                                         wp8[:, c, :],
                                         start=(c == 0), stop=(c == n_ic0 - 1))
                for t in range(T):
                    dst = ys[t][:, ob * OBW:(ob + 1) * OBW]
                    nc.scalar.activation(dst, ps[t][:], AF.Copy,
                                         bias=0.0, scale=c_cur[t][:])

            # drain remaining L1 quant runs, then trigger its gather
            for h1 in l1_run_iter:
                _emit_quant_run(1, h1, swb[1])
            _gather_layer(1)

            # drain any remaining abs reduces
            for nxt in abs_iter:
                li_, wr_, rl_, c_ = nxt
                nc.vector.tensor_reduce(partials[:, c_:c_ + 1], wr_[:],
                                        mybir.AxisListType.X, OP.add,
                                        apply_absolute_value=True)

            # ======== P6: AR23 ========
            pm23 = small.tile([P, 2], F32, tag="pm23", name="pm23")
            for i, li in enumerate((2, 3)):
                hi = col if li == 3 else cols[3]
                nc.vector.tensor_reduce(pm23[:, i:i + 1],
                                        partials[:, cols[li]:hi],
                                        mybir.AxisListType.X, OP.add)
            _papr_ar(pm23[:], ar_in23, ar_out23, 2, "ar23")
            pms23 = small.tile([P, 2], F32, tag="pms23", name="pms23")
            nc.gpsimd.dma_start(pms23[:], ar_out23[:])

            # ======== P7: L0 tail + remaining quant + main L1-3 ========
            def _tail(li, ys, bns, c_list):
                """LN+SiLU (fused, hw table) + act quant + transpose."""
                L = layers[li]
                dout = L["dout"]
                n_ic_next = layers[li + 1]["n_ic"]
                QW = min(2048, dout)
                xqT_next, c_next = [], []
                for t in range(T):
                    if li == 0:  # bn_stats deferred (DVE was quantizing)
                        for ob in range(L["n_ob"]):
                            nc.vector.bn_stats(
                                bns[t][:, ob * 6:(ob + 1) * 6],
                                ys[t][:, ob * OBW:(ob + 1) * OBW])
                    mv = small.tile([P, 2], F32, tag="mv", name=f"mv{li}_{t}")
                    nc.vector.bn_aggr(mv[:], bns[t][:])
                    v = small.tile([P, 1], F32, tag="vvar", name=f"v{li}_{t}")
                    nc.vector.tensor_scalar(v[:], mv[:, 1:2], float(EPS), None,
                                            OP.add)
                    istd = _rsqrt_newton(nc, small, v[:])
                    nmi = small.tile([P, 1], F32, tag="nmi", name=f"nmi{li}_{t}")
                    nc.vector.scalar_tensor_tensor(nmi[:], mv[:, 0:1], -1.0,
                                                   istd[:], OP.mult, OP.mult)
                    nc.scalar.activation(ys[t][:], ys[t][:], AF.Silu,
                                         bias=nmi[:], scale=istd[:])
                    am = small.tile([P, 1], F32, tag="amn", name=f"am{li}_{t}")
                    nc.vector.tensor_reduce(am[:], ys[t][:],
                                            mybir.AxisListType.X, OP.max,
                                            apply_absolute_value=True)
                    nc.vector.tensor_scalar(am[:], am[:], float(EPS), None,
                                            OP.max)
                    sc = small.tile([P, 1], F32, tag="scn", name=f"sc{li}_{t}")
                    nc.vector.tensor_scalar(sc[:], am[:], 1.0 / 127.0, None,
                                            OP.mult)
                    nc.vector.reciprocal(sc[:], sc[:])
                    cn = small.tile([P, 1], F32, tag=f"c{li + 1}_{t}",
                                    name=f"c{li + 1}_{t}")
                    nc.vector.scalar_tensor_tensor(cn[:], am[:], 1.0 / 127.0,
                                                   mwb[li + 1][:],
                                                   OP.mult, OP.mult)
                    c_next.append(cn)
                    xqn = xqnpool.tile([P, dout], BF16, tag="xqn",
                                       name=f"xqn{li}_{t}")
                    xT = xqTpool.tile([P, n_ic_next, P], BF16, tag="xqT",
                                      name=f"xT{li}_{t}")
                    icq = QW // P
                    for ch in range(dout // QW):
                        uu = upool.tile([P, QW], F32, tag="u",
                                        name=f"ur{li}_{t}_{ch}")
                        nc.scalar.activation(uu[:],
                                             ys[t][:, ch * QW:(ch + 1) * QW],
                                             AF.Copy, bias=MAGIC, scale=sc[:])
                        nc.vector.tensor_scalar(xqn[:, ch * QW:(ch + 1) * QW],
                                                uu[:], MAGIC, None, OP.subtract)
                        # transpose each finished half immediately
                        nc.sync.dma_start_transpose(
                            xT[:, ch * icq:(ch + 1) * icq, :],
                            xqn[:, ch * QW:(ch + 1) * QW])
                    xqT_next.append(xT)
                return xqT_next, c_next

            xqT_cur, c_cur = _tail(0, ys, bns, c_cur)

            # L1 scale for next dequant was computed in P4; L2/L3 swl now.
            # (AR23 lands well before the DVE reaches this point.)
            mwb[2], swb[2] = _swl_chain(2, pms23[:, 0:1])
            mwb[3], swb[3] = _swl_chain(3, pms23[:, 1:2])
            _quant_layer_coop(2, swb[2])
            _quant_layer_coop(3, swb[3])

            for L in layers[1:]:
                li, n_ic, n_ob = L["li"], L["n_ic"], L["n_ob"]
                panel_ic, n_panels = L["panel_ic"], L["n_panels"]
                dout, dreal = L["dout"], L["dreal"]
                is_last = (li == 3)

                ys = [ypool.tile([P, dreal], F32, tag="y", name=f"y{li}_{t}")
                      for t in range(T)]
                bns = [small.tile([P, n_ob * 6], F32, tag=f"bn{t}",
                                  name=f"bn{li}_{t}")
                       for t in range(T)] if not is_last else None

                for ob in range(n_ob):
                    ow = L["ob_w"][ob]
                    ps = [psum.tile([P, OBW], F32, tag="ps",
                                    name=f"ps{li}_{ob}_{t}") for t in range(T)]
                    for panel in range(n_panels):
                        wp = wpool.tile([P, panel_ic, OBW], FP8, tag="wp",
                                        name=f"wp{li}_{ob}_{panel}")
                        g0 = (ob * n_ic + panel * panel_ic)
                        uoff = g0 * CH_ELS
                        nc.sync.dma_start(
                            wp[:], image[li][uoff:uoff + panel_ic * CH_ELS]
                            .rearrange("(p c f) -> p c f", p=P, c=panel_ic))
                        for t in range(T):
                            for cc in range(panel_ic):
                                c = panel * panel_ic + cc
                                nc.tensor.matmul(
                                    ps[t][:], xqT_cur[t][:, c, :],
                                    wp[:, cc, :],
                                    start=(c == 0), stop=(c == n_ic - 1))
                    for t in range(T):
                        dst = ys[t][:, ob * OBW:ob * OBW + ow]
                        if not is_last:
                            nc.scalar.activation(dst, ps[t][:, :ow], AF.Copy,
                                                 bias=0.0, scale=c_cur[t][:])
                            nc.vector.bn_stats(bns[t][:, ob * 6:(ob + 1) * 6],
                                               dst)
                        else:
                            nc.scalar.activation(dst, ps[t][:, :ow], AF.Sigmoid,
                                                 bias=0.0, scale=c_cur[t][:])

                if is_last:
                    for t in range(T):
                        mzt = upool.tile([P, OBINS], F32, tag="u",
                                           name=f"mzt{t}")
                        nc.vector.tensor_scalar(mzt[:], ys[t][:, 0:OBINS],
                                                float(OBINS - 1), 1.0,
                                                OP.mult, OP.add)
                        nc.scalar.dma_start(mz_out[t * P:(t + 1) * P, :], mzt[:])
                        iit = upool.tile([P, OBINS], F32, tag="u",
                                           name=f"iit{t}")
                        nc.vector.tensor_scalar(iit[:], ys[t][:, OBINS:2 * OBINS],
                                                100.0, None, OP.mult)
                        nc.scalar.dma_start(ii_out[t * P:(t + 1) * P, :], iit[:])
                    continue

                xqT_cur, c_cur = _tail(li, ys, bns, c_cur)

    nc.compile()
    return nc


def prepare_inputs(cfg, x, W0, W1, W2, W3):
    """Host-side sharding: per-core input maps. Weight chunks are shipped
    unit-major: unit u = (layer, ob, panel) is a [128, panel_ic*512] block,
    rows = partitions, contiguous per row; chunk cc of the unit holds
    W_l[ob*512+o, (panel*panel_ic+cc)*128+p] at [p, cc*512+o] (i.e. W^T).
    W0 additionally ships as a full ob-major image (same for all cores) for
    the local layer-0 quantize."""
    layers, total_ch, per_rank, T = _plan(cfg)
    b_core = T * P
    Ws = [np.asarray(W0), np.asarray(W1), np.asarray(W2), np.asarray(W3)]
    WTs = []
    for L, W in zip(layers, Ws):
        WT = np.zeros((L["din"], L["dout"]), dtype=np.float32)
        WT[:, :L["dreal"]] = W.T
        WTs.append(WT)

    shards = [np.empty(per_rank * CH_ELS, dtype=np.float32)
              for _ in range(N_CORES)]
    for L in layers:
        li, pr = L["li"], L["per_rank"]
        n_ic, panel_ic = L["n_ic"], L["panel_ic"]
        WT = WTs[li]
        for r in range(N_CORES):
            g0 = r * pr
            dst = shards[r]
            for j in range(0, pr, panel_ic):
                g = g0 + j
                ob, ic0 = divmod(g, n_ic)
                assert ic0 % panel_ic == 0
                blk = WT[ic0 * P:(ic0 + panel_ic) * P,
                         ob * OBW:(ob + 1) * OBW]          # [panel_ic*128, 512]
                blk = blk.reshape(panel_ic, P, OBW).transpose(1, 0, 2)
                off = (L["ch_base"] // N_CORES + j) * CH_ELS
                dst[off:off + panel_ic * CH_ELS] = blk.reshape(-1)

    # full W0 image, ob-major, grouped per quant run of Q0RUN chunks
    L0 = layers[0]
    n_ic0, n_ob0 = L0["n_ic"], L0["n_ob"]
    q0run = min(4, n_ic0)
    WT0 = WTs[0]
    w0f = np.empty(L0["n_ch"] * CH_ELS, dtype=np.float32)
    pos = 0
    for ob in range(n_ob0):
        for r0 in range(0, n_ic0, q0run):
            blk = WT0[r0 * P:(r0 + q0run) * P, ob * OBW:(ob + 1) * OBW]
            blk = blk.reshape(q0run, P, OBW).transpose(1, 0, 2)
            w0f[pos:pos + q0run * CH_ELS] = blk.reshape(-1)
            pos += q0run * CH_ELS

    x = np.asarray(x, dtype=np.float32)
    in_maps = []
    for r in range(N_CORES):
        in_maps.append(dict(
            xs=np.ascontiguousarray(x[r * b_core:(r + 1) * b_core]),
            wsh=shards[r],
            w0f=w0f,
        ))
    return in_maps


_NC_CACHE = {}


def _get_nc(cfg_key):
    if cfg_key not in _NC_CACHE:
        _NC_CACHE[cfg_key] = build(dict(cfg_key))
    return _NC_CACHE[cfg_key]


def run(cfg, x, W0, W1, W2, W3, trace=False):
    layers, total_ch, per_rank, T = _plan(cfg)
    b_core = T * P
    nc = _get_nc(tuple(sorted(cfg.items())))
    in_maps = prepare_inputs(cfg, x, W0, W1, W2, W3)
    res = run_bass_kernel_spmd(nc, in_maps, core_ids=list(range(N_CORES)),
                               trace=trace)
    mz = np.concatenate([res.results[r]["mz"] for r in range(N_CORES)], axis=0)
    ii = np.concatenate([res.results[r]["ii"] for r in range(N_CORES)], axis=0)
    return (mz, ii), res


def kernel(x, W0, W1, W2, W3, g0, b0, g1, b1, g2, b2):
    """Full-input entry point. g/b are identity (ones/zeros) in this problem's
    setup; LayerNorm affine is a no-op and is validated here."""
    for g in (g0, g1, g2):
        assert np.allclose(np.asarray(g), 1.0), "non-identity LN gain unsupported"
    for b in (b0, b1, b2):
        assert np.allclose(np.asarray(b), 0.0), "non-zero LN bias unsupported"
    (mz, ii), _ = run(FULL_CFG, x, W0, W1, W2, W3, trace=False)
    return (mz, ii)
